# revision 53
# baseline (speedup 1.0000x reference)
"""Trainium2 Bass kernel for GQA attention with RoPE (B=2, S=1024, HID=2048,
16 q heads / 4 kv heads, head dim 128, causal).

Sharding: 8 cores = 2 batches x 4 kv-head groups. Core c = b*4 + g handles
batch b and kv head g (query heads 4g..4g+3). Each core computes a partial
output y_part = attn_heads @ wo_shard; the host sums the 4 partials per batch.

All tensors fp16 on the wire and in SBUF (host casts inputs; host upcasts and
sums the fp16 partials; rel err ~5e-4 vs the fp32 reference). All matmuls
fp16 (1 cyc/row). fp8 was tried and abandoned: exp values span e^{+-9.2},
outside e4m3's range, and fp8-denominator-only fails on softmax-concentrated
queries (no error cancellation against the fp16 numerator).

Per-core dataflow:
  Phase A (per 128-row chunk g, software-pipelined; x streams on the SP DGE
  queue, wq/wkv/cossin on the ACT queue — the host concatenates wk||wv and
  cos||sin so every DMA has >=512B descriptor runs; consts on Pool; proj is
  emitted before the next tp so the in-order PE queue never blocks on an
  x DMA while weights are resident):
    x chunk --PE transpose--> xT --mm--> q,k,v (natural); batched RoPE on DVE
    (broadcast cos/sin over the 5 q/k blocks); PE transpose q_rope/k_rope ->
    persistent qT[d,h,s], kT[d,s]; v natural -> vv[s,d].
  Attention (per 256-col tile t and HEAD PAIR hp, 2-stage pipelined with
  512-wide moving operands; one step per phase-A iteration fills PE bubbles,
  the bulk + all wo run after phase A, when the qkv/tp PSUM banks are
  released to the wo/ud2 rings):
    scoresT[sk,(2h,sq)] = kT_chunk.T @ qT_pair ; +m12 causal mask broadcast
    over the pair (DVE) ; exp on ACT -> expst f16 ; den = ones.T @ expst ;
    U^T accumulated fp16 ; rec = 1/den (DVE) ; uT = U^T * rec (DVE, f16).
    wo: y[g,:] = sum_h uT_h.T @ wo_h -> y_sb f16 -> DRAM (one DMA per row).
"""

import sys

import numpy as np

for _p in ("/opt/trn_rl_repo", "/root/.axon_site/_ro/trn_rl_repo"):
    if _p not in sys.path:
        sys.path.append(_p)

from contextlib import ExitStack

import concourse.bass as bass
import concourse.mybir as mybir
from concourse import bacc
from concourse.masks import make_identity
from concourse.tile import TileContext

P = 128           # partitions / head dim / seq chunk
S = 1024          # sequence length
HID = 2048        # model dim
NH = 4            # query heads per core
D = 128           # head dim
TQ = 256          # query macro-tile
NT = S // TQ      # 4 macro tiles
KC = HID // P     # 16 contraction chunks
NSK = S // P      # 8 key chunks
NG = S // P       # 8 row chunks
H2 = D // 2
F32 = mybir.dt.float32
F16 = mybir.dt.float16
F8 = mybir.dt.float8e4
SCALE = 1.0 / float(np.sqrt(D))
NEG = -30000.0
AL = mybir.AluOpType
AF = mybir.ActivationFunctionType
DR = mybir.MatmulPerfMode.DoubleRow

# fp8 expst is dead: exp values span e^{+-9.2}, outside e4m3 range (den can
# round to 0 -> 1/den=inf -> NaN; large values saturate at 448)
USE_F8 = False

N_CORES = 8
B = 2
N_KV = 4


def build_nc():
    nc = bacc.Bacc("TRN2", target_bir_lowering=False, debug=False)
    # all inputs arrive HOST-PRE-REARRANGED into partition-major SBUF
    # layout: DMA descriptor runs become 4-16KB (the DMA engines round-robin
    # descriptors across queues, so big runs = proportionally more bandwidth)
    x_d = nc.declare_dram_parameter("x", [P, NG * HID], F16, isOutput=False)
    cossin_d = nc.declare_dram_parameter(
        "cossin", [P, NG * 2 * D], F16, isOutput=False
    )
    wq_d = nc.declare_dram_parameter("wq", [P, KC * NH * D], F16, isOutput=False)
    wkv_d = nc.declare_dram_parameter("wkv", [P, KC * 2 * D], F16, isOutput=False)
    wo_d = nc.declare_dram_parameter("wo", [P, NH * HID], F16, isOutput=False)
    out_d = nc.declare_dram_parameter("out", [S, HID], F16, isOutput=True)

    with TileContext(nc) as tc, ExitStack() as ctx:
        consts = ctx.enter_context(tc.tile_pool(name="consts", bufs=1))
        wpool = ctx.enter_context(tc.tile_pool(name="wpool", bufs=1))
        persist = ctx.enter_context(tc.tile_pool(name="persist", bufs=1))

        # ---- tile declarations (DMAs can start before consts are built) ----
        ident_f32 = consts.tile([P, P], F32, tag="ident_f32")
        ident = consts.tile([P, P], F16, tag="ident")
        ones8 = consts.tile([P, 2, P], F8, tag="ones8")
        ones16 = consts.tile([P, P], F16, tag="ones16")

        # ---- weights (partition-chunked layouts) ----
        wq_sb = wpool.tile([P, KC, NH * D], F16, tag="wq")
        wq_r = wq_d[:].rearrange("p (c n) -> p c n", n=NH * D)
        wkv_sb = wpool.tile([P, KC, 2 * D], F16, tag="wkv")
        wo_sb = wpool.tile([P, NH, HID], F16, tag="wo")
        wo_r = wo_d[:].rearrange("p (h n) -> p h n", n=HID)
        cossin_sb = wpool.tile([P, NG, 2 * D], F16, tag="cossin")

        # persistent transposed activations
        qT_all = persist.tile([P, NH, S], F16, tag="qT")   # [d, h, sq]
        kT = persist.tile([P, S], F16, tag="kT")           # [d, sk]
        vv = persist.tile([P, NSK, D], F16, tag="vv")      # v natural [sk, d]

        # ---- SBUF working pools ----
        pa = ctx.enter_context(tc.tile_pool(name="pa", bufs=2))
        pb = ctx.enter_context(tc.tile_pool(name="pb", bufs=2))
        # ---- PSUM (8 banks): phase A uses qkv 2 + tp 2 (inner stack,
        # released before wo); s 2 + ud 2 persist; y 2 allocated after ----
        ps_s = ctx.enter_context(tc.tile_pool(name="ps_s", bufs=2, space="PSUM"))
        ps_ud = ctx.enter_context(tc.tile_pool(name="ps_ud", bufs=1, space="PSUM"))
        phase_a_ctx = ExitStack()
        ps_qkv = phase_a_ctx.enter_context(
            tc.tile_pool(name="ps_qkv", bufs=1, space="PSUM")
        )
        ps_tp = phase_a_ctx.enter_context(
            tc.tile_pool(name="ps_tp", bufs=2, space="PSUM")
        )

        x_tiles = [None] * NG
        x_r = x_d[:].rearrange("p (c n) -> p c n", n=HID)

        def emit_xdma_pair(g, single=False, queue=None):
            """One DMA per chunk pair (fewer per-DMA overheads)."""
            n = 1 if single else 2
            xp = pa.tile([P, n, HID], F16, tag=f"xnat{g}", bufs=1)
            (queue or nc.sync).dma_start(out=xp, in_=x_r[:, g : g + n, :])
            for j in range(n):
                x_tiles[g + j] = xp[:, j, :]

        # warm up the PE clock immediately — the stationary only needs a
        # cheap DVE memset, not the gpsimd-built identity
        warm_src = consts.tile([P, P], F16, tag="warmsrc")
        nc.vector.memset(warm_src, 1.0)
        warm_ps = ps_s.tile([P, 2 * TQ], F32, tag="s", name="warm")
        for _ in range(12):
            nc.tensor.matmul(
                warm_ps[:, 0:P], warm_src, warm_src, start=True, stop=True
            )

        # Parallel DGE queues: x + wq on sync (SP, HWDGE — costs no engine
        # time); wkv/cos/sin on the scalar queue; Pool keeps only the const
        # builds (ident/m12) plus wo and output DMAs.
        # one SERIAL priority queue (scalar DGE) for the early-critical
        # stream, in exact need order — cross-queue descriptor round-robin
        # would otherwise starve whichever stream has smaller descriptors.
        # x45/x67 go on sync, emitted mid-loop so they start late.
        x0 = pa.tile([P, HID], F16, tag="xnat0", bufs=1)
        nc.scalar.dma_start(out=x0[:, 0 : HID // 2], in_=x_r[:, 0, 0 : HID // 2])
        nc.scalar.dma_start(out=x0[:, HID // 2 :], in_=x_r[:, 0, HID // 2 :])
        x_tiles[0] = x0
        nc.scalar.dma_start(out=wq_sb[:, 0:8, :], in_=wq_r[:, 0:8, :])
        emit_xdma_pair(1, single=True, queue=nc.scalar)
        nc.scalar.dma_start(out=wq_sb[:, 8:16, :], in_=wq_r[:, 8:16, :])
        nc.scalar.dma_start(
            out=wkv_sb, in_=wkv_d[:].rearrange("p (c n) -> p c n", n=2 * D)
        )
        emit_xdma_pair(2, queue=nc.scalar)
        nc.scalar.dma_start(
            out=cossin_sb,
            in_=cossin_d[:].rearrange("p (c d) -> p c d", d=2 * D),
        )

        # ---- constants (Pool engine work overlapping the DMAs) ----
        make_identity(nc, ident_f32)
        nc.vector.tensor_copy(ident, ident_f32)
        warm_drain = pa.tile([P, 4], F32, tag="warmdrain", bufs=1)
        nc.vector.tensor_copy(warm_drain, warm_ps[:, 0:4])
        nc.vector.memset(ones8, 1.0)
        nc.vector.memset(ones16, 1.0)
        # causal masks for the two diagonal-straddling chunk positions
        m12 = consts.tile([P, 2 * TQ], F32, tag="m12")
        nc.gpsimd.memset(m12, 0.0)
        nc.gpsimd.affine_select(
            out=m12[:, 0:TQ], in_=m12[:, 0:TQ], compare_op=AL.is_ge, fill=NEG,
            base=0, pattern=[[1, TQ]], channel_multiplier=-1,
        )
        nc.gpsimd.affine_select(
            out=m12[:, TQ : 2 * TQ], in_=m12[:, TQ : 2 * TQ],
            compare_op=AL.is_ge, fill=NEG,
            base=-P, pattern=[[1, TQ]], channel_multiplier=-1,
        )
        emit_xdma_pair(4)
        wo_next = [0]

        def emit_wo_dma():
            h = wo_next[0]
            if h < NH:
                nc.gpsimd.dma_start(
                    out=wo_sb[:, h : h + 2, :], in_=wo_r[:, h : h + 2, :]
                )
                wo_next[0] += 2

        def transposes(g):
            """x chunk -> xT (PE transpose, f16)."""
            x_nat = x_tiles[g]
            xT = pa.tile([P, KC, P], F16, tag="xT", bufs=2)
            xT_flat = xT.rearrange("p c d -> p (c d)")
            for kb in range(KC // 4):
                tp_ps = ps_tp.tile([P, 5 * P], F16, tag="tp", name="tp")
                for j in range(4):
                    k = 4 * kb + j
                    nc.tensor.transpose(
                        tp_ps[:, j * P : (j + 1) * P],
                        x_nat[:, k * P : (k + 1) * P],
                        ident,
                    )
                if kb % 2 == 0:
                    nc.vector.tensor_copy(
                        xT_flat[:, kb * 4 * P : (kb + 1) * 4 * P], tp_ps[:, 0 : 4 * P]
                    )
                else:
                    nc.scalar.activation(
                        out=xT_flat[:, kb * 4 * P : (kb + 1) * 4 * P],
                        in_=tp_ps[:, 0 : 4 * P],
                        func=AF.Copy,
                    )
            return xT

        def proj(g, xT):
            """q, k, v projections for chunk g (PE, accumulating in PSUM)."""
            qkv_ps = ps_qkv.tile([P, NH * D + 2 * D], F32, tag="qkv")
            q_ps = qkv_ps[:, 0 : NH * D]
            kv_ps = qkv_ps[:, NH * D : NH * D + 2 * D]
            for k in range(KC):
                nc.tensor.matmul(
                    q_ps, xT[:, k, :], wq_sb[:, k, :],
                    start=(k == 0), stop=(k == KC - 1),
                )
            for k in range(KC):
                nc.tensor.matmul(
                    kv_ps, xT[:, k, :], wkv_sb[:, k, :],
                    start=(k == 0), stop=(k == KC - 1),
                )
            # copy-out split across ACT (q) and DVE (kv) to free the bank fast
            qkv_sb = pa.tile([P, NH * D + 2 * D], F16, tag="qkvsb")
            nc.scalar.activation(
                out=qkv_sb[:, 0 : NH * D], in_=q_ps, func=AF.Copy
            )
            nc.vector.tensor_copy(qkv_sb[:, NH * D :], kv_ps)
            return qkv_sb

        def rope_stage(g, qkv_sb):
            """Batched RoPE over the 5 q/k blocks (DVE, broadcast cos/sin)."""
            qk = qkv_sb[:, 0 : 5 * D].rearrange("p (f d) -> p f d", d=D)
            sin_lo = cossin_sb[:, g : g + 1, D : D + H2].to_broadcast((P, 5, H2))
            sin_hi = cossin_sb[:, g : g + 1, D + H2 : 2 * D].to_broadcast((P, 5, H2))
            cos_bc = cossin_sb[:, g : g + 1, 0:D].to_broadcast((P, 5, D))
            tmp = pa.tile([P, 5, D], F16, tag="ropetmp")
            dst = pa.tile([P, 5, D], F16, tag="qkrope")
            nc.vector.scalar_tensor_tensor(
                out=tmp[:, :, 0:H2], in0=qk[:, :, H2:D], scalar=-1.0,
                in1=sin_lo, op0=AL.mult, op1=AL.mult,
            )
            nc.vector.tensor_tensor(
                out=tmp[:, :, H2:D], in0=qk[:, :, 0:H2], in1=sin_hi, op=AL.mult
            )
            nc.vector.tensor_tensor(out=dst, in0=qk, in1=cos_bc, op=AL.mult)
            nc.vector.tensor_tensor(
                out=dst.rearrange("p f d -> p (f d)"),
                in0=dst.rearrange("p f d -> p (f d)"),
                in1=tmp.rearrange("p f d -> p (f d)"),
                op=AL.add,
            )
            # v copy-out (cast f16)
            nc.vector.tensor_copy(vv[:, g, :], qkv_sb[:, 5 * D : 6 * D])
            return dst

        def rope_transpose(g, dst):
            """Transpose RoPE'd q/k into persistent qT_all / kT."""
            tq_ps = ps_tp.tile([P, 5 * P], F16, tag="tp", name="tq")
            for f in range(5):
                nc.tensor.transpose(
                    tq_ps[:, f * P : (f + 1) * P], dst[:, f, :], ident
                )
            nc.vector.tensor_copy(
                qT_all[:, :, g * P : (g + 1) * P],
                tq_ps[:, 0 : 4 * P].rearrange("p (h d) -> p h d", h=NH),
            )
            nc.scalar.activation(
                out=kT[:, g * P : (g + 1) * P], in_=tq_ps[:, 4 * P : 5 * P],
                func=AF.Copy,
            )

        ropes = [None] * NG
        pend = [None] * NG

        def emit_phase_a(g):
            if g >= 2:
                gg = g - 2
                with nc.named_scope(f"rope_{gg}"):
                    ropes[gg] = rope_stage(gg, pend[gg][1])
            if g >= 1 and g - 1 < NG:
                gg = g - 1
                with nc.named_scope(f"proj_{gg}"):
                    qkv_sb = proj(gg, pend[gg][0])
                pend[gg][1] = qkv_sb
            if g < NG:
                if g == 1:
                    emit_xdma_pair(6)
                if g in (2, 3):
                    emit_wo_dma()
                with nc.named_scope(f"tp_{g}"):
                    xT = transposes(g)
                pend[g] = [xT, None]
            if g >= 2:
                gg = g - 2
                with nc.named_scope(f"ropeT_{gg}"):
                    rope_transpose(gg, ropes[gg])
                pend[gg] = None

        # ---------- attention ----------
        EDT = F8 if USE_F8 else F16

        # ud is a 2-bank tile (u 512 + den 512 f32); a second 1-buf pool is
        # allocated post-phase-A so the ring deepens to 2 without exceeding
        # the 8-bank budget during phase A
        ps_extra = [None]
        ps_s2 = [None]
        ud_i = [0]
        s_i = [0]

        def s_tile():
            s_i[0] += 1
            if ps_s2[0] is not None and s_i[0] % 3 == 0:
                return ps_s2[0].tile([P, 2 * TQ], F32, tag="s2", name="s")
            return ps_s.tile([P, 2 * TQ], F32, tag="s", name="s")

        def ud_tile():
            ud_i[0] += 1
            if ps_extra[0] is not None and ud_i[0] % 2 == 0:
                return ps_extra[0].tile([P, 4, TQ], F32, tag="ud2", name="ud")
            return ps_ud.tile([P, 4, TQ], F32, tag="ud", name="ud")

        def scores_head(t, hp):
            """scoresT for a head PAIR (moving dim 512) + mask + exp."""
            qT_2h = qT_all[:, hp : hp + 2, t * TQ : (t + 1) * TQ]
            expst = pb.tile([P, NSK, 2, TQ], EDT, tag="expst", bufs=6)
            for ik in range(2 * (t + 1)):
                s_ps = s_tile().rearrange("p (o f) -> p o f", f=TQ)
                nc.tensor.matmul(
                    s_ps, kT[:, ik * P : (ik + 1) * P], qT_2h,
                    start=True, stop=True,
                )
                if ik >= 2 * t:
                    half = ik - 2 * t
                    m12b = m12[:, half * TQ : (half + 1) * TQ].rearrange(
                        "p (o f) -> p o f", o=1
                    ).to_broadcast((P, 2, TQ))
                    nc.vector.tensor_tensor(
                        out=s_ps, in0=s_ps, in1=m12b, op=AL.add
                    )
                nc.scalar.activation(
                    out=expst[:, ik, :, :], in_=s_ps, func=AF.Exp, scale=SCALE,
                )
            return expst

        def dnpv_head(t, hp, expst, uT_t):
            """denominator + PV matmuls for a head pair, normalize (DVE)."""
            nsk = 2 * (t + 1)
            ud_ps = ud_tile()
            u_ps = ud_ps[:, 0:2, :]
            den_ps = ud_ps[:, 2:4, :]
            for ik in range(nsk):
                nc.tensor.matmul(
                    den_ps, ones16, expst[:, ik, :, :],
                    start=(ik == 0), stop=(ik == nsk - 1),
                )
            rec = pb.tile([P, 2, TQ], F32, tag="rec", bufs=2)
            nc.vector.reciprocal(
                rec.rearrange("p o f -> p (o f)"),
                den_ps.rearrange("p o f -> p (o f)"),
            )
            for ik in range(nsk):
                nc.tensor.matmul(
                    u_ps, vv[:, ik, :], expst[:, ik, :, :],
                    start=(ik == 0), stop=(ik == nsk - 1),
                )
            nc.vector.tensor_tensor(
                out=uT_t[:, hp : hp + 2, :], in0=u_ps, in1=rec, op=AL.mult
            )

        ps_y_box = [None]

        def wo_stage(t, uT_t):
            for sub in range(2):
                g = 2 * t + sub
                y_sb = pb.tile([P, HID], F16, tag="ysb", bufs=2)
                for n in range(HID // 512):
                    y_ps = ps_y_box[0].tile([P, 512], F32, tag="y", name="y")
                    for h in range(NH):
                        nc.tensor.matmul(
                            y_ps,
                            uT_t[:, h, sub * P : (sub + 1) * P],
                            wo_sb[:, h, n * 512 : (n + 1) * 512],
                            start=(h == 0), stop=(h == NH - 1),
                        )
                    if n % 2 == 0:
                        nc.vector.tensor_copy(
                            y_sb[:, n * 512 : (n + 1) * 512], y_ps
                        )
                    else:
                        nc.scalar.activation(
                            out=y_sb[:, n * 512 : (n + 1) * 512], in_=y_ps,
                            func=AF.Copy,
                        )
                nc.gpsimd.dma_start(
                    out=out_d[g * P : (g + 1) * P, :], in_=y_sb
                )

        steps = [(t, hp) for t in range(NT) for hp in (0, 2)]
        uts = {}
        sc_i = [0]
        dn_i = [0]
        pending_wo = []

        def emit_sc():
            if sc_i[0] >= len(steps):
                return False
            t, hp = steps[sc_i[0]]
            if hp == 0:
                uts[t] = pb.tile([P, NH, TQ], F16, tag="uT", name=f"uT{t}", bufs=4)
            with nc.named_scope(f"sc_{t}_{hp}"):
                uts[(t, hp)] = scores_head(t, hp)
            sc_i[0] += 1
            return True

        def emit_dn():
            if dn_i[0] >= sc_i[0] or dn_i[0] >= len(steps):
                return False
            t, hp = steps[dn_i[0]]
            with nc.named_scope(f"dnpv_{t}_{hp}"):
                dnpv_head(t, hp, uts.pop((t, hp)), uts[t])
            dn_i[0] += 1
            if hp == 2:
                pending_wo.append(t)
            return True

        def emit_wo():
            if not pending_wo:
                return False
            t = pending_wo.pop(0)
            with nc.named_scope(f"wo_{t}"):
                wo_stage(t, uts.pop(t))
            return True

        # drive: phase A strictly prioritized; sc-ONLY doses fill PE
        # bubbles (dnpv needs the ud ring + DVE, both contended in phase A)
        done_g = [-1]
        for g in range(NG + 2):
            emit_phase_a(g)
            done_g[0] = g - 2
            if g >= 3 and sc_i[0] < len(steps) and sc_i[0] - dn_i[0] < 4:
                t, _hp = steps[sc_i[0]]
                if 2 * t + 2 <= done_g[0]:
                    emit_sc()
        # phase A fully emitted: release its PSUM banks, give wo its own
        # ring and deepen the ud ring
        phase_a_ctx.close()
        ps_y_box[0] = ctx.enter_context(
            tc.tile_pool(name="ps_y", bufs=1, space="PSUM")
        )
        ps_extra[0] = ctx.enter_context(
            tc.tile_pool(name="ps_extra", bufs=1, space="PSUM")
        )
        ps_s2[0] = ctx.enter_context(
            tc.tile_pool(name="ps_s2", bufs=1, space="PSUM")
        )
        # steady state: keep sc THREE steps ahead of dnpv so the ACT exp
        # chain (5-7us for big tiles) never stalls the PE; wo after each tile
        while True:
            p = emit_sc()
            if sc_i[0] >= len(steps) or sc_i[0] - dn_i[0] >= 3:
                p = emit_dn() or p
            p = emit_wo() or p
            if not p:
                break

    nc.compile()
    return nc


def _pmajor(a, p=P):
    """[C*p, n] -> [p, C*n] partition-major (matches 'p c n' SBUF tiles)."""
    cn = a.shape[0] // p
    return np.ascontiguousarray(
        a.reshape(cn, p, -1).transpose(1, 0, 2).reshape(p, -1), dtype=np.float16
    )


def shard_inputs(x, cos, sin, wq, wk, wv, wo):
    """Build per-core input maps (fp16, partition-major): core = b*4 + g."""
    in_maps = []
    cossin = _pmajor(np.concatenate([cos, sin], axis=1))
    for c in range(N_CORES):
        b, g = divmod(c, N_KV)
        in_maps.append(
            {
                "x": _pmajor(x[b]),
                "cossin": cossin,
                "wq": _pmajor(wq[:, g * NH * D : (g + 1) * NH * D]),
                "wkv": _pmajor(
                    np.concatenate(
                        [wk[:, g * D : (g + 1) * D], wv[:, g * D : (g + 1) * D]],
                        axis=1,
                    )
                ),
                "wo": _pmajor(wo[g * NH * D : (g + 1) * NH * D, :]),
            }
        )
    return in_maps


_NC_CACHE = {}


def get_nc():
    if "nc" not in _NC_CACHE:
        _NC_CACHE["nc"] = build_nc()
    return _NC_CACHE["nc"]


def kernel(x, cos, sin, wq, wk, wv, wo, _trace=False):
    from concourse.bass_utils import run_bass_kernel_spmd

    x = np.asarray(x, dtype=np.float32)
    cos = np.asarray(cos, dtype=np.float32)
    sin = np.asarray(sin, dtype=np.float32)
    wq = np.asarray(wq, dtype=np.float32)
    wk = np.asarray(wk, dtype=np.float32)
    wv = np.asarray(wv, dtype=np.float32)
    wo = np.asarray(wo, dtype=np.float32)

    nc = get_nc()
    in_maps = shard_inputs(x, cos, sin, wq, wk, wv, wo)
    res = run_bass_kernel_spmd(nc, in_maps, list(range(N_CORES)), trace=_trace)
    parts = [
        np.asarray(res.results[c]["out"], dtype=np.float32) for c in range(N_CORES)
    ]
    y = np.stack(
        [sum(parts[b * N_KV + g] for g in range(N_KV)) for b in range(B)], axis=0
    )
    if _trace:
        kernel.last_result = res
    return y


# revision 54
# speedup vs baseline: 1.0696x; 1.0696x over previous
"""Trainium2 Bass kernel for GQA attention with RoPE (B=2, S=1024, HID=2048,
16 q heads / 4 kv heads, head dim 128, causal).

Sharding: 8 cores = 2 batches x 4 kv-head groups. Core c = b*4 + g handles
batch b and kv head g (query heads 4g..4g+3). Each core computes a partial
output y_part = attn_heads @ wo_shard; the host sums the 4 partials per batch.

All tensors fp16 on the wire and in SBUF (host casts inputs; host upcasts and
sums the fp16 partials; rel err ~5e-4 vs the fp32 reference). All matmuls
fp16 (1 cyc/row). fp8 was tried and abandoned: exp values span e^{+-9.2},
outside e4m3's range, and fp8-denominator-only fails on softmax-concentrated
queries (no error cancellation against the fp16 numerator).

Per-core dataflow:
  Phase A (per 128-row chunk g, software-pipelined; x streams on the SP DGE
  queue, wq/wkv/cossin on the ACT queue — the host concatenates wk||wv and
  cos||sin so every DMA has >=512B descriptor runs; consts on Pool; proj is
  emitted before the next tp so the in-order PE queue never blocks on an
  x DMA while weights are resident):
    x chunk --PE transpose--> xT --mm--> q,k,v (natural); batched RoPE on DVE
    (broadcast cos/sin over the 5 q/k blocks); PE transpose q_rope/k_rope ->
    persistent qT[d,h,s], kT[d,s]; v natural -> vv[s,d].
  Attention (per 256-col tile t and HEAD PAIR hp, 2-stage pipelined with
  512-wide moving operands; one step per phase-A iteration fills PE bubbles,
  the bulk + all wo run after phase A, when the qkv/tp PSUM banks are
  released to the wo/ud2 rings):
    scoresT[sk,(2h,sq)] = kT_chunk.T @ qT_pair ; +m12 causal mask broadcast
    over the pair (DVE) ; exp on ACT -> expst f16 ; den = ones.T @ expst ;
    U^T accumulated fp16 ; rec = 1/den (DVE) ; uT = U^T * rec (DVE, f16).
    wo: y[g,:] = sum_h uT_h.T @ wo_h -> y_sb f16 -> DRAM (one DMA per row).
"""

import sys

import numpy as np

for _p in ("/opt/trn_rl_repo", "/root/.axon_site/_ro/trn_rl_repo"):
    if _p not in sys.path:
        sys.path.append(_p)

from contextlib import ExitStack

import concourse.bass as bass
import concourse.mybir as mybir
from concourse import bacc
from concourse.masks import make_identity
from concourse.tile import TileContext

P = 128           # partitions / head dim / seq chunk
S = 1024          # sequence length
HID = 2048        # model dim
NH = 4            # query heads per core
D = 128           # head dim
TQ = 256          # query macro-tile
NT = S // TQ      # 4 macro tiles
KC = HID // P     # 16 contraction chunks
NSK = S // P      # 8 key chunks
NG = S // P       # 8 row chunks
H2 = D // 2
F32 = mybir.dt.float32
F16 = mybir.dt.float16
F8 = mybir.dt.float8e4
SCALE = 1.0 / float(np.sqrt(D))
NEG = -30000.0
AL = mybir.AluOpType
AF = mybir.ActivationFunctionType
DR = mybir.MatmulPerfMode.DoubleRow

# fp8 expst is dead: exp values span e^{+-9.2}, outside e4m3 range (den can
# round to 0 -> 1/den=inf -> NaN; large values saturate at 448)
USE_F8 = False

N_CORES = 8
B = 2
N_KV = 4


def build_nc():
    nc = bacc.Bacc("TRN2", target_bir_lowering=False, debug=False)
    # all inputs arrive HOST-PRE-REARRANGED into partition-major SBUF
    # layout: DMA descriptor runs become 4-16KB (the DMA engines round-robin
    # descriptors across queues, so big runs = proportionally more bandwidth)
    x_d = nc.declare_dram_parameter("x", [P, NG * HID], F16, isOutput=False)
    cossin_d = nc.declare_dram_parameter(
        "cossin", [P, NG * 2 * D], F16, isOutput=False
    )
    wq_d = nc.declare_dram_parameter("wq", [P, KC * NH * D], F16, isOutput=False)
    wkv_d = nc.declare_dram_parameter("wkv", [P, KC * 2 * D], F16, isOutput=False)
    wo_d = nc.declare_dram_parameter("wo", [P, NH * HID], F16, isOutput=False)
    out_d = nc.declare_dram_parameter("out", [S, HID], F16, isOutput=True)

    with TileContext(nc) as tc, ExitStack() as ctx:
        consts = ctx.enter_context(tc.tile_pool(name="consts", bufs=1))
        wpool = ctx.enter_context(tc.tile_pool(name="wpool", bufs=1))
        persist = ctx.enter_context(tc.tile_pool(name="persist", bufs=1))

        # ---- tile declarations (DMAs can start before consts are built) ----
        ident_f32 = consts.tile([P, P], F32, tag="ident_f32")
        ident = consts.tile([P, P], F16, tag="ident")
        ones8 = consts.tile([P, 2, P], F8, tag="ones8")
        ones16 = consts.tile([P, P], F16, tag="ones16")

        # ---- weights (partition-chunked layouts) ----
        wq_sb = wpool.tile([P, KC, NH * D], F16, tag="wq")
        wq_r = wq_d[:].rearrange("p (c n) -> p c n", n=NH * D)
        wkv_sb = wpool.tile([P, KC, 2 * D], F16, tag="wkv")
        wo_sb = wpool.tile([P, NH, HID], F16, tag="wo")
        wo_r = wo_d[:].rearrange("p (h n) -> p h n", n=HID)
        cossin_sb = wpool.tile([P, NG, 2 * D], F16, tag="cossin")

        # persistent transposed activations
        qT_all = persist.tile([P, NH, S], F16, tag="qT")   # [d, h, sq]
        kT = persist.tile([P, S], F16, tag="kT")           # [d, sk]
        vv = persist.tile([P, NSK, D], F16, tag="vv")      # v natural [sk, d]

        # ---- SBUF working pools ----
        pa = ctx.enter_context(tc.tile_pool(name="pa", bufs=2))
        pb = ctx.enter_context(tc.tile_pool(name="pb", bufs=2))
        # ---- PSUM (8 banks): phase A uses qkv 2 + tp 2 (inner stack,
        # released before wo); s 2 + ud 2 persist; y 2 allocated after ----
        ps_s = ctx.enter_context(tc.tile_pool(name="ps_s", bufs=2, space="PSUM"))
        ps_ud = ctx.enter_context(tc.tile_pool(name="ps_ud", bufs=1, space="PSUM"))
        phase_a_ctx = ExitStack()
        ps_qkv = phase_a_ctx.enter_context(
            tc.tile_pool(name="ps_qkv", bufs=1, space="PSUM")
        )
        ps_tp = phase_a_ctx.enter_context(
            tc.tile_pool(name="ps_tp", bufs=2, space="PSUM")
        )

        x_tiles = [None] * NG
        x_r = x_d[:].rearrange("p (c n) -> p c n", n=HID)

        def emit_xdma_pair(g, single=False, queue=None):
            """One DMA per chunk pair (fewer per-DMA overheads)."""
            n = 1 if single else 2
            xp = pa.tile([P, n, HID], F16, tag=f"xnat{g}", bufs=1)
            (queue or nc.sync).dma_start(out=xp, in_=x_r[:, g : g + n, :])
            for j in range(n):
                x_tiles[g + j] = xp[:, j, :]

        # warm up the PE clock immediately — the stationary only needs a
        # cheap DVE memset, not the gpsimd-built identity
        warm_src = consts.tile([P, P], F16, tag="warmsrc")
        nc.vector.memset(warm_src, 1.0)
        warm_ps = ps_s.tile([P, 2 * TQ], F32, tag="s", name="warm")
        for _ in range(12):
            nc.tensor.matmul(
                warm_ps[:, 0:P], warm_src, warm_src, start=True, stop=True
            )

        # Parallel DGE queues: x + wq on sync (SP, HWDGE — costs no engine
        # time); wkv/cos/sin on the scalar queue; Pool keeps only the const
        # builds (ident/m12) plus wo and output DMAs.
        # one SERIAL priority queue (scalar DGE) for the early-critical
        # stream, in exact need order — cross-queue descriptor round-robin
        # would otherwise starve whichever stream has smaller descriptors.
        # x45/x67 go on sync, emitted mid-loop so they start late.
        x0 = pa.tile([P, HID], F16, tag="xnat0", bufs=1)
        nc.scalar.dma_start(out=x0[:, 0 : HID // 2], in_=x_r[:, 0, 0 : HID // 2])
        nc.scalar.dma_start(out=x0[:, HID // 2 :], in_=x_r[:, 0, HID // 2 :])
        x_tiles[0] = x0
        nc.scalar.dma_start(out=wq_sb[:, 0:8, :], in_=wq_r[:, 0:8, :])
        emit_xdma_pair(1, single=True, queue=nc.scalar)
        nc.scalar.dma_start(out=wq_sb[:, 8:16, :], in_=wq_r[:, 8:16, :])
        nc.scalar.dma_start(
            out=wkv_sb, in_=wkv_d[:].rearrange("p (c n) -> p c n", n=2 * D)
        )
        emit_xdma_pair(2, queue=nc.scalar)
        nc.scalar.dma_start(
            out=cossin_sb,
            in_=cossin_d[:].rearrange("p (c d) -> p c d", d=2 * D),
        )

        # ---- constants (Pool engine work overlapping the DMAs) ----
        make_identity(nc, ident_f32)
        nc.vector.tensor_copy(ident, ident_f32)
        warm_drain = pa.tile([P, 4], F32, tag="warmdrain", bufs=1)
        nc.vector.tensor_copy(warm_drain, warm_ps[:, 0:4])
        nc.vector.memset(ones8, 1.0)
        nc.vector.memset(ones16, 1.0)
        # causal masks for the two diagonal-straddling chunk positions
        m12 = consts.tile([P, 2 * TQ], F32, tag="m12")
        nc.gpsimd.memset(m12, 0.0)
        nc.gpsimd.affine_select(
            out=m12[:, 0:TQ], in_=m12[:, 0:TQ], compare_op=AL.is_ge, fill=NEG,
            base=0, pattern=[[1, TQ]], channel_multiplier=-1,
        )
        nc.gpsimd.affine_select(
            out=m12[:, TQ : 2 * TQ], in_=m12[:, TQ : 2 * TQ],
            compare_op=AL.is_ge, fill=NEG,
            base=-P, pattern=[[1, TQ]], channel_multiplier=-1,
        )
        emit_xdma_pair(4)
        wo_next = [0]

        def emit_wo_dma():
            h = wo_next[0]
            if h < NH:
                nc.gpsimd.dma_start(
                    out=wo_sb[:, h : h + 2, :], in_=wo_r[:, h : h + 2, :]
                )
                wo_next[0] += 2

        def transposes(g):
            """x chunk -> xT (PE transpose, f16)."""
            x_nat = x_tiles[g]
            xT = pa.tile([P, KC, P], F16, tag="xT", bufs=2)
            xT_flat = xT.rearrange("p c d -> p (c d)")
            for kb in range(KC // 4):
                tp_ps = ps_tp.tile([P, 5 * P], F16, tag="tp", name="tp")
                for j in range(4):
                    k = 4 * kb + j
                    nc.tensor.transpose(
                        tp_ps[:, j * P : (j + 1) * P],
                        x_nat[:, k * P : (k + 1) * P],
                        ident,
                    )
                if kb % 2 == 0:
                    nc.vector.tensor_copy(
                        xT_flat[:, kb * 4 * P : (kb + 1) * 4 * P], tp_ps[:, 0 : 4 * P]
                    )
                else:
                    nc.scalar.activation(
                        out=xT_flat[:, kb * 4 * P : (kb + 1) * 4 * P],
                        in_=tp_ps[:, 0 : 4 * P],
                        func=AF.Copy,
                    )
            return xT

        def proj(g, xT):
            """q, k, v projections for chunk g (PE, accumulating in PSUM)."""
            qkv_ps = ps_qkv.tile([P, NH * D + 2 * D], F32, tag="qkv")
            q_ps = qkv_ps[:, 0 : NH * D]
            kv_ps = qkv_ps[:, NH * D : NH * D + 2 * D]
            for k in range(KC):
                nc.tensor.matmul(
                    q_ps, xT[:, k, :], wq_sb[:, k, :],
                    start=(k == 0), stop=(k == KC - 1),
                )
            for k in range(KC):
                nc.tensor.matmul(
                    kv_ps, xT[:, k, :], wkv_sb[:, k, :],
                    start=(k == 0), stop=(k == KC - 1),
                )
            # copy-out split across ACT (q) and DVE (kv) to free the bank fast
            qkv_sb = pa.tile([P, NH * D + 2 * D], F16, tag="qkvsb")
            nc.scalar.activation(
                out=qkv_sb[:, 0 : NH * D], in_=q_ps, func=AF.Copy
            )
            nc.vector.tensor_copy(qkv_sb[:, NH * D :], kv_ps)
            return qkv_sb

        def rope_stage(g, qkv_sb):
            """Batched RoPE over the 5 q/k blocks (DVE, broadcast cos/sin)."""
            qk = qkv_sb[:, 0 : 5 * D].rearrange("p (f d) -> p f d", d=D)
            sin_lo = cossin_sb[:, g : g + 1, D : D + H2].to_broadcast((P, 5, H2))
            sin_hi = cossin_sb[:, g : g + 1, D + H2 : 2 * D].to_broadcast((P, 5, H2))
            cos_bc = cossin_sb[:, g : g + 1, 0:D].to_broadcast((P, 5, D))
            tmp = pa.tile([P, 5, D], F16, tag="ropetmp")
            dst = pa.tile([P, 5, D], F16, tag="qkrope")
            nc.vector.scalar_tensor_tensor(
                out=tmp[:, :, 0:H2], in0=qk[:, :, H2:D], scalar=-1.0,
                in1=sin_lo, op0=AL.mult, op1=AL.mult,
            )
            nc.vector.tensor_tensor(
                out=tmp[:, :, H2:D], in0=qk[:, :, 0:H2], in1=sin_hi, op=AL.mult
            )
            nc.vector.tensor_tensor(out=dst, in0=qk, in1=cos_bc, op=AL.mult)
            nc.vector.tensor_tensor(
                out=dst.rearrange("p f d -> p (f d)"),
                in0=dst.rearrange("p f d -> p (f d)"),
                in1=tmp.rearrange("p f d -> p (f d)"),
                op=AL.add,
            )
            # v copy-out (cast f16)
            nc.vector.tensor_copy(vv[:, g, :], qkv_sb[:, 5 * D : 6 * D])
            return dst

        def rope_transpose(g, dst):
            """Transpose RoPE'd q/k into persistent qT_all / kT."""
            tq_ps = ps_tp.tile([P, 5 * P], F16, tag="tp", name="tq")
            for f in range(5):
                nc.tensor.transpose(
                    tq_ps[:, f * P : (f + 1) * P], dst[:, f, :], ident
                )
            nc.vector.tensor_copy(
                qT_all[:, :, g * P : (g + 1) * P],
                tq_ps[:, 0 : 4 * P].rearrange("p (h d) -> p h d", h=NH),
            )
            nc.scalar.activation(
                out=kT[:, g * P : (g + 1) * P], in_=tq_ps[:, 4 * P : 5 * P],
                func=AF.Copy,
            )

        ropes = [None] * NG
        pend = [None] * NG

        def emit_phase_a(g):
            if g >= 2:
                gg = g - 2
                with nc.named_scope(f"rope_{gg}"):
                    ropes[gg] = rope_stage(gg, pend[gg][1])
            if g >= 1 and g - 1 < NG:
                gg = g - 1
                with nc.named_scope(f"proj_{gg}"):
                    qkv_sb = proj(gg, pend[gg][0])
                pend[gg][1] = qkv_sb
            if g < NG:
                if g == 1:
                    emit_xdma_pair(6)
                if g in (2, 3):
                    emit_wo_dma()
                with nc.named_scope(f"tp_{g}"):
                    xT = transposes(g)
                pend[g] = [xT, None]
            if g >= 2:
                gg = g - 2
                with nc.named_scope(f"ropeT_{gg}"):
                    rope_transpose(gg, ropes[gg])
                pend[gg] = None

        # ---------- attention ----------
        EDT = F8 if USE_F8 else F16

        # ud is a 2-bank tile (u 512 + den 512 f32); a second 1-buf pool is
        # allocated post-phase-A so the ring deepens to 2 without exceeding
        # the 8-bank budget during phase A
        ps_extra = [None]
        ud_i = [0]

        def s_tile():
            return ps_s.tile([P, 2 * TQ], F32, tag="s", name="s")

        def ud_tile():
            ud_i[0] += 1
            if ps_extra[0] is not None and ud_i[0] % 2 == 0:
                return ps_extra[0].tile([P, 4, TQ], F32, tag="ud2", name="ud")
            return ps_ud.tile([P, 4, TQ], F32, tag="ud", name="ud")

        def scores_head(t, hp):
            """scoresT for a head PAIR (moving dim 512) + mask + exp."""
            qT_2h = qT_all[:, hp : hp + 2, t * TQ : (t + 1) * TQ]
            expst = pb.tile([P, NSK, 2, TQ], EDT, tag="expst", bufs=6)
            for ik in range(2 * (t + 1)):
                s_ps = s_tile().rearrange("p (o f) -> p o f", f=TQ)
                nc.tensor.matmul(
                    s_ps, kT[:, ik * P : (ik + 1) * P], qT_2h,
                    start=True, stop=True,
                )
                if ik >= 2 * t:
                    half = ik - 2 * t
                    m12b = m12[:, half * TQ : (half + 1) * TQ].rearrange(
                        "p (o f) -> p o f", o=1
                    ).to_broadcast((P, 2, TQ))
                    nc.vector.tensor_tensor(
                        out=s_ps, in0=s_ps, in1=m12b, op=AL.add
                    )
                nc.scalar.activation(
                    out=expst[:, ik, :, :], in_=s_ps, func=AF.Exp, scale=SCALE,
                )
            return expst

        def dnpv_head(t, hp, expst, uT_t):
            """denominator + PV matmuls for a head pair, normalize (DVE)."""
            nsk = 2 * (t + 1)
            ud_ps = ud_tile()
            u_ps = ud_ps[:, 0:2, :]
            den_ps = ud_ps[:, 2:4, :]
            for ik in range(nsk):
                nc.tensor.matmul(
                    den_ps, ones16, expst[:, ik, :, :],
                    start=(ik == 0), stop=(ik == nsk - 1),
                )
            rec = pb.tile([P, 2, TQ], F32, tag="rec", bufs=2)
            nc.vector.reciprocal(
                rec.rearrange("p o f -> p (o f)"),
                den_ps.rearrange("p o f -> p (o f)"),
            )
            for ik in range(nsk):
                nc.tensor.matmul(
                    u_ps, vv[:, ik, :], expst[:, ik, :, :],
                    start=(ik == 0), stop=(ik == nsk - 1),
                )
            nc.vector.tensor_tensor(
                out=uT_t[:, hp : hp + 2, :], in0=u_ps, in1=rec, op=AL.mult
            )

        ps_y_box = [None]

        def wo_stage(t, uT_t):
            for sub in range(2):
                g = 2 * t + sub
                y_sb = pb.tile([P, HID], F16, tag="ysb", bufs=2)
                for n in range(HID // 512):
                    y_ps = ps_y_box[0].tile([P, 512], F32, tag="y", name="y")
                    for h in range(NH):
                        nc.tensor.matmul(
                            y_ps,
                            uT_t[:, h, sub * P : (sub + 1) * P],
                            wo_sb[:, h, n * 512 : (n + 1) * 512],
                            start=(h == 0), stop=(h == NH - 1),
                        )
                    if n % 2 == 0:
                        nc.vector.tensor_copy(
                            y_sb[:, n * 512 : (n + 1) * 512], y_ps
                        )
                    else:
                        nc.scalar.activation(
                            out=y_sb[:, n * 512 : (n + 1) * 512], in_=y_ps,
                            func=AF.Copy,
                        )
                nc.gpsimd.dma_start(
                    out=out_d[g * P : (g + 1) * P, :], in_=y_sb
                )

        steps = [(t, hp) for t in range(NT) for hp in (0, 2)]
        uts = {}
        sc_i = [0]
        dn_i = [0]
        pending_wo = []

        def emit_sc():
            if sc_i[0] >= len(steps):
                return False
            t, hp = steps[sc_i[0]]
            if hp == 0:
                uts[t] = pb.tile([P, NH, TQ], F16, tag="uT", name=f"uT{t}", bufs=4)
            with nc.named_scope(f"sc_{t}_{hp}"):
                uts[(t, hp)] = scores_head(t, hp)
            sc_i[0] += 1
            return True

        def emit_dn():
            if dn_i[0] >= sc_i[0] or dn_i[0] >= len(steps):
                return False
            t, hp = steps[dn_i[0]]
            with nc.named_scope(f"dnpv_{t}_{hp}"):
                dnpv_head(t, hp, uts.pop((t, hp)), uts[t])
            dn_i[0] += 1
            if hp == 2:
                pending_wo.append(t)
            return True

        def emit_wo():
            if not pending_wo:
                return False
            t = pending_wo.pop(0)
            with nc.named_scope(f"wo_{t}"):
                wo_stage(t, uts.pop(t))
            return True

        # drive: phase A strictly prioritized; sc-ONLY doses fill PE
        # bubbles (dnpv needs the ud ring + DVE, both contended in phase A)
        done_g = [-1]
        for g in range(NG + 2):
            emit_phase_a(g)
            done_g[0] = g - 2
            if g >= 3 and sc_i[0] < len(steps) and sc_i[0] - dn_i[0] < 4:
                t, _hp = steps[sc_i[0]]
                if 2 * t + 2 <= done_g[0]:
                    emit_sc()
        # phase A fully emitted: release its PSUM banks, give wo its own
        # ring and deepen the ud ring
        phase_a_ctx.close()
        ps_y_box[0] = ctx.enter_context(
            tc.tile_pool(name="ps_y", bufs=2, space="PSUM")
        )
        ps_extra[0] = ctx.enter_context(
            tc.tile_pool(name="ps_extra", bufs=1, space="PSUM")
        )
        # steady state: keep sc THREE steps ahead of dnpv so the ACT exp
        # chain (5-7us for big tiles) never stalls the PE; wo after each tile
        while True:
            p = emit_sc()
            if sc_i[0] >= len(steps) or sc_i[0] - dn_i[0] >= 3:
                p = emit_dn() or p
            p = emit_wo() or p
            if not p:
                break

    nc.compile()
    return nc


def _pmajor(a, p=P):
    """[C*p, n] -> [p, C*n] partition-major (matches 'p c n' SBUF tiles)."""
    cn = a.shape[0] // p
    return np.ascontiguousarray(
        a.reshape(cn, p, -1).transpose(1, 0, 2).reshape(p, -1), dtype=np.float16
    )


def shard_inputs(x, cos, sin, wq, wk, wv, wo):
    """Build per-core input maps (fp16, partition-major): core = b*4 + g."""
    in_maps = []
    cossin = _pmajor(np.concatenate([cos, sin], axis=1))
    for c in range(N_CORES):
        b, g = divmod(c, N_KV)
        in_maps.append(
            {
                "x": _pmajor(x[b]),
                "cossin": cossin,
                "wq": _pmajor(wq[:, g * NH * D : (g + 1) * NH * D]),
                "wkv": _pmajor(
                    np.concatenate(
                        [wk[:, g * D : (g + 1) * D], wv[:, g * D : (g + 1) * D]],
                        axis=1,
                    )
                ),
                "wo": _pmajor(wo[g * NH * D : (g + 1) * NH * D, :]),
            }
        )
    return in_maps


_NC_CACHE = {}


def get_nc():
    if "nc" not in _NC_CACHE:
        _NC_CACHE["nc"] = build_nc()
    return _NC_CACHE["nc"]


def kernel(x, cos, sin, wq, wk, wv, wo, _trace=False):
    from concourse.bass_utils import run_bass_kernel_spmd

    x = np.asarray(x, dtype=np.float32)
    cos = np.asarray(cos, dtype=np.float32)
    sin = np.asarray(sin, dtype=np.float32)
    wq = np.asarray(wq, dtype=np.float32)
    wk = np.asarray(wk, dtype=np.float32)
    wv = np.asarray(wv, dtype=np.float32)
    wo = np.asarray(wo, dtype=np.float32)

    nc = get_nc()
    in_maps = shard_inputs(x, cos, sin, wq, wk, wv, wo)
    res = run_bass_kernel_spmd(nc, in_maps, list(range(N_CORES)), trace=_trace)
    parts = [
        np.asarray(res.results[c]["out"], dtype=np.float32) for c in range(N_CORES)
    ]
    y = np.stack(
        [sum(parts[b * N_KV + g] for g in range(N_KV)) for b in range(B)], axis=0
    )
    if _trace:
        kernel.last_result = res
    return y
